# revision 5
# baseline (speedup 1.0000x reference)
"""AttentiveStatsPooling Trainium2 kernel.

Full-input contract: kernel(**inputs) takes the unsharded numpy inputs
  x            (32, 1536, 2048) f32
  padding_mask (32, 2048)       bool
  W_tdnn       (128, 1536)      f32
  b_tdnn       (128,)           f32
  W_attn       (1536, 128)      f32
  b_attn       (1536,)          f32
and returns the full (32, 3072) f32 output.

Sharding: data-parallel over batch. 8 cores x 4 samples each, weights
replicated.

Key algorithmic move: masked positions contribute EXACTLY zero (the
reference's exp(a - 1e9 - rowmax) underflows to 0.0 in f32), so the host
compacts each sample's time axis to its ~1024 unmasked positions and
zero-pads to a fixed Tp (multiple of 128, 1152 for the seed-0 dataset).
Pad positions have x=0, so e_pad = tanh(b_tdnn) and their logit
a_pad[c] = sum_o W_attn^T[o,c] tanh(b_tdnn)[o] is a per-channel constant;
their S1/S2 contribution is zero and their S0 contribution kb*exp(a_pad)
is precomputed on the host (with the same bf16 weights the device uses)
and subtracted in the tail. This removes the mask matmuls entirely and
scales all per-element engine work by Tp/T ~ 0.56.

Math per sample (all t below over the compacted axis):
  e    = tanh(W_tdnn @ x + b_tdnn)         (BN, Tp)
  a    = W_attn @ e   (b_attn dropped: constant along t, cancels in softmax)
  S0   = sum_t exp(a) - kb*E_pad;  S1 = sum_t exp(a)*x;  S2 = sum_t exp(a)*x^2
  mean = S1/S0;  std = sqrt(clip(S2/S0 - mean^2, 1e-9))

Engine assignment (HW-microbenchmarked per [128,2048] tile: DVE TT mult
bf16 930ns (2x mode), DVE reduce 1365ns, ACT exp+accum 2265ns, ACT
copy+accum 2210ns, Pool TT mult 5393ns):
  ACT:  exp (accum_out gives S0 free) + tanh + A2/12 of the S2 reduces
        (Copy+accum)
  DVE:  p1 = exp*x (+S1 reduce), p2 = p1*x for (12-Q)/12 chunks, the rest
        of the S2 reduces
  Pool: p2 for Q/12 chunks (otherwise idle engine, takes ~1/3 of the
        product work at ~1/3 efficiency)
  PE:   mm1, mm2 (no mask matmuls)
"""

import math

import numpy as np
import ml_dtypes

B, C, T = 32, 1536, 2048
BN = 128
NCORES = 8
SPC = B // NCORES  # samples per core
CK = C // 128      # c chunks of 128 partitions

BF16 = ml_dtypes.bfloat16

# tuning: per sample (12 chunks), Q chunks' p2 on Pool, A2 chunks' S2 on ACT
Q_POOL = 8
A2_ACT = 7

_PROG_CACHE = {}
_LAST_TP = [1152]


def _jblocks(Tp):
    out = []
    off = 0
    while off < Tp:
        w = min(512, Tp - off)
        out.append((off, w))
        off += w
    return out


def _build_program(reps=None, Tp=None):
    """Build the per-core program. reps=None: straight-line body.
    reps=K: wrap the whole body in a hardware For_i loop (timing only)."""
    import concourse.bacc as bacc
    import concourse.tile as tile
    import concourse.mybir as mybir
    from contextlib import nullcontext
    from concourse.bass_interp import get_hw_module

    if Tp is None:
        Tp = _LAST_TP[0]

    dt = mybir.dt
    AF = mybir.ActivationFunctionType
    OP = mybir.AluOpType
    JB = _jblocks(Tp)

    nc = bacc.Bacc(
        "TRN2",
        target_bir_lowering=False,
        debug=False,
        num_devices=NCORES,
        num_swdge_queues=4,
    )
    x_d = nc.dram_tensor("x", [SPC, C, Tp], dt.bfloat16, kind="ExternalInput")
    wt_d = nc.dram_tensor("wt", [C, BN], dt.bfloat16, kind="ExternalInput")
    wa_d = nc.dram_tensor("wa", [BN, C], dt.bfloat16, kind="ExternalInput")
    bt_d = nc.dram_tensor("bt", [BN, 1], dt.float32, kind="ExternalInput")
    kbe_d = nc.dram_tensor("kbe", [SPC, 128, CK], dt.float32, kind="ExternalInput")
    out_d = nc.dram_tensor("out", [SPC, 2 * C], dt.float32, kind="ExternalOutput")

    with tile.TileContext(nc) as tc:
        with (
            tc.tile_pool(name="const", bufs=1) as constp,
            tc.tile_pool(name="xin", bufs=3 * CK) as xp,
            tc.tile_pool(name="esb", bufs=3) as ep,
            tc.tile_pool(name="expm", bufs=6) as xpm,
            tc.tile_pool(name="prod", bufs=10) as prp,
            tc.tile_pool(name="junk", bufs=3) as jkp,
            tc.tile_pool(name="stats", bufs=1) as statsp,
            tc.tile_pool(name="tail", bufs=2) as tailp,
            tc.tile_pool(name="pse", bufs=2, space="PSUM") as psep,
            tc.tile_pool(name="pa", bufs=2, space="PSUM") as psp,
        ):
            # ---- constants ------------------------------------------------
            wt_sb = constp.tile([128, CK, BN], dt.bfloat16, tag="wt")
            nc.sync.dma_start(
                out=wt_sb, in_=wt_d.ap().rearrange("(k p) o -> p k o", p=128)
            )
            wa_sb = constp.tile([128, C], dt.bfloat16, tag="wa")
            nc.sync.dma_start(out=wa_sb, in_=wa_d.ap())
            bt_sb = constp.tile([128, 1], dt.float32, tag="bt")
            nc.sync.dma_start(out=bt_sb, in_=bt_d.ap())
            kbe_sb = constp.tile([128, SPC, CK], dt.float32, tag="kbe")
            nc.sync.dma_start(
                out=kbe_sb, in_=kbe_d.ap().rearrange("s p k -> p s k")
            )

            loop_cm = tc.For_i(0, reps, 1) if reps is not None else nullcontext()
            with loop_cm:
                stats = []
                for s in range(SPC):
                    S0 = statsp.tile([128, CK], dt.float32, tag=f"S0_{s}")
                    S1 = statsp.tile([128, CK], dt.float32, tag=f"S1_{s}")
                    S2 = statsp.tile([128, CK], dt.float32, tag=f"S2_{s}")
                    stats.append((S0, S1, S2))

                # process samples in pairs; the two chunk streams interleave
                # so the engines always have an independent chunk in flight
                for s0 in range(0, SPC, 2):
                    pair = [s0, s0 + 1]
                    xts = {}
                    esbs = {}
                    for s in pair:
                        for k in range(CK):
                            xt = xp.tile(
                                [128, Tp], dt.bfloat16, tag="x", name=f"x_{s}_{k}"
                            )
                            nc.sync.dma_start(
                                out=xt, in_=x_d.ap()[s, k * 128 : (k + 1) * 128, :]
                            )
                            xts[(s, k)] = xt

                    # mm1 + tanh for both samples of the pair, one 512-block
                    # of PSUM at a time (keeps 3 full-width pa bufs free)
                    for s in pair:
                        e_sb = ep.tile([128, Tp], dt.bfloat16, tag="e", name=f"e_{s}")
                        for bi, (off, w) in enumerate(JB):
                            pse = psep.tile(
                                [128, 512], dt.float32, tag="pse",
                                name=f"pse_{s}_{bi}",
                            )
                            for k in range(CK):
                                nc.tensor.matmul(
                                    pse[:, 0:w],
                                    lhsT=wt_sb[:, k, :],
                                    rhs=xts[(s, k)][:, off : off + w],
                                    start=(k == 0),
                                    stop=(k == CK - 1),
                                )
                            nc.scalar.activation(
                                out=e_sb[:, off : off + w],
                                in_=pse[:, 0:w],
                                func=AF.Tanh,
                                bias=bt_sb,
                                scale=1.0,
                            )
                        esbs[s] = e_sb

                    def s2_stage(s, c, p2):
                        # S2[c] reduce: ACT Copy+accum for A2_ACT of 12
                        # chunks, DVE reduce for the rest
                        if (c % 12) % 2 == 0 or (c % 12) == 1:
                            if A2_ACT >= 7:
                                on_act = True
                            else:
                                on_act = (c % 12) < 2 * A2_ACT
                        else:
                            on_act = False
                        if on_act:
                            junk = jkp.tile(
                                [128, Tp], dt.bfloat16, tag="junk",
                                name=f"junk_{s}_{c}",
                            )
                            nc.scalar.activation(
                                out=junk,
                                in_=p2,
                                func=AF.Copy,
                                accum_out=stats[s][2][:, c : c + 1],
                            )
                        else:
                            nc.vector.tensor_reduce(
                                out=stats[s][2][:, c : c + 1],
                                in_=p2,
                                op=OP.add,
                                axis=mybir.AxisListType.X,
                            )

                    pending = []
                    for c in range(CK):
                        for s in pair:
                            S0, S1, S2 = stats[s]
                            e_sb = esbs[s]
                            expm = xpm.tile(
                                [128, Tp], dt.bfloat16, tag="expm",
                                name=f"expm_{s}_{c}",
                            )
                            pa = psp.tile(
                                [128, Tp], dt.float32, tag="ps", name=f"pa_{s}_{c}"
                            )
                            for off, w in JB:
                                nc.tensor.matmul(
                                    pa[:, off : off + w],
                                    lhsT=wa_sb[:, c * 128 : (c + 1) * 128],
                                    rhs=e_sb[:, off : off + w],
                                    start=True,
                                    stop=True,
                                )
                            nc.scalar.activation(
                                out=expm,
                                in_=pa,
                                func=AF.Exp,
                                accum_out=S0[:, c : c + 1],
                            )
                            p1 = prp.tile(
                                [128, Tp], dt.bfloat16, tag="p1", name=f"p1_{s}_{c}"
                            )
                            nc.vector.tensor_tensor(
                                out=p1, in0=expm, in1=xts[(s, c)], op=OP.mult
                            )
                            nc.vector.tensor_reduce(
                                out=S1[:, c : c + 1],
                                in_=p1,
                                op=OP.add,
                                axis=mybir.AxisListType.X,
                            )
                            p2 = prp.tile(
                                [128, Tp], dt.bfloat16, tag="p2", name=f"p2_{s}_{c}"
                            )
                            # p2 on Pool for Q_POOL of 12 chunks, else DVE
                            if (c % 3 != 2) if Q_POOL == 8 else (c % 12 < Q_POOL):
                                nc.gpsimd.tensor_tensor(
                                    out=p2, in0=p1, in1=xts[(s, c)], op=OP.mult
                                )
                            else:
                                nc.vector.tensor_tensor(
                                    out=p2, in0=p1, in1=xts[(s, c)], op=OP.mult
                                )
                            pending.append((s, c, p2))
                            if len(pending) > 4:
                                s2_stage(*pending.pop(0))
                    for item in pending:
                        s2_stage(*item)

                # ---- tail: mean/std + output DMA --------------------------
                for s in range(SPC):
                    S0, S1, S2 = stats[s]
                    s0c = tailp.tile([128, CK], dt.float32, tag="s0c", name=f"s0c_{s}")
                    nc.vector.tensor_tensor(
                        out=s0c, in0=S0, in1=kbe_sb[:, s, :], op=OP.subtract
                    )
                    r0 = tailp.tile([128, CK], dt.float32, tag="r0", name=f"r0_{s}")
                    nc.vector.reciprocal(out=r0, in_=s0c)
                    mean = tailp.tile(
                        [128, CK], dt.float32, tag="mean", name=f"mean_{s}"
                    )
                    nc.vector.tensor_tensor(out=mean, in0=S1, in1=r0, op=OP.mult)
                    ex2 = tailp.tile([128, CK], dt.float32, tag="ex2", name=f"ex2_{s}")
                    nc.vector.tensor_tensor(out=ex2, in0=S2, in1=r0, op=OP.mult)
                    m2 = tailp.tile([128, CK], dt.float32, tag="m2", name=f"m2_{s}")
                    nc.vector.tensor_tensor(out=m2, in0=mean, in1=mean, op=OP.mult)
                    var = tailp.tile([128, CK], dt.float32, tag="var", name=f"var_{s}")
                    nc.vector.tensor_tensor(out=var, in0=ex2, in1=m2, op=OP.subtract)
                    nc.vector.tensor_scalar(
                        out=var,
                        in0=var,
                        scalar1=1e-9,
                        scalar2=None,
                        op0=OP.max,
                    )
                    std = tailp.tile([128, CK], dt.float32, tag="std", name=f"std_{s}")
                    nc.scalar.activation(out=std, in_=var, func=AF.Sqrt)
                    nc.sync.dma_start(
                        out=out_d.ap()[s, 0:C].rearrange("(ck p) -> p ck", p=128),
                        in_=mean,
                    )
                    nc.sync.dma_start(
                        out=out_d.ap()[s, C : 2 * C].rearrange(
                            "(ck p) -> p ck", p=128
                        ),
                        in_=std,
                    )

    nc.compile()
    nc.m = get_hw_module(nc.m)
    return nc


def _get_program(Tp):
    key = ("nc", Tp)
    if key not in _PROG_CACHE:
        _PROG_CACHE[key] = _build_program(Tp=Tp)
    return _PROG_CACHE[key]


def _prep_inputs(x, padding_mask, W_tdnn, b_tdnn, W_attn, b_attn):
    """Host-side prep: cast/compact/transpose, build per-core input maps."""
    keep = ~np.asarray(padding_mask)
    n_b = keep.sum(axis=1).astype(np.int64)
    Tp = max(512, int(math.ceil(n_b.max() / 128.0) * 128))
    _LAST_TP[0] = Tp

    xb = np.ascontiguousarray(x).astype(BF16)
    xv = xb.view(np.uint16)
    xc = np.zeros((B, C, Tp), dtype=np.uint16)
    for b in range(B):
        idx = np.flatnonzero(keep[b])
        xc[b, :, : idx.size] = xv[b][:, idx]
    xc = xc.view(BF16)

    wt = np.ascontiguousarray(W_tdnn.T).astype(BF16)  # (C, BN)
    wa = np.ascontiguousarray(W_attn.T).astype(BF16)  # (BN, C)
    bt = np.ascontiguousarray(b_tdnn.astype(np.float32).reshape(BN, 1))

    # pad-correction: kb * exp(a_pad) with the same bf16 weights the device
    # uses (pads have x=0 -> e_pad = tanh(b_tdnn))
    e_pad = np.tanh(bt.astype(np.float64))[:, 0].astype(BF16)  # (BN,)
    a_pad = wa.astype(np.float64).T @ e_pad.astype(np.float64)  # (C,)
    E = np.exp(a_pad)  # (C,)
    kb = (Tp - n_b).astype(np.float64)  # (B,)
    # kbe[s, p, k] = kb[s] * E[k*128 + p]
    Em = E.reshape(CK, 128).T  # (128, CK)
    kbe = (kb[:, None, None] * Em[None, :, :]).astype(np.float32)  # (B,128,CK)

    in_maps = []
    for i in range(NCORES):
        sl = slice(i * SPC, (i + 1) * SPC)
        in_maps.append(
            {
                "x": np.ascontiguousarray(xc[sl]),
                "wt": wt,
                "wa": wa,
                "bt": bt,
                "kbe": np.ascontiguousarray(kbe[sl]),
            }
        )
    return in_maps


def kernel(x, padding_mask, W_tdnn, b_tdnn, W_attn, b_attn):
    from concourse.bass_utils import run_bass_kernel_spmd

    in_maps = _prep_inputs(x, padding_mask, W_tdnn, b_tdnn, W_attn, b_attn)
    nc = _get_program(_LAST_TP[0])
    res = run_bass_kernel_spmd(nc, in_maps, core_ids=list(range(NCORES)))
    out = np.concatenate([res.results[i]["out"] for i in range(NCORES)], axis=0)
    return out.astype(np.float32)


# revision 7
# speedup vs baseline: 1.1245x; 1.1245x over previous
"""AttentiveStatsPooling Trainium2 kernel.

Full-input contract: kernel(**inputs) takes the unsharded numpy inputs
  x            (32, 1536, 2048) f32
  padding_mask (32, 2048)       bool
  W_tdnn       (128, 1536)      f32
  b_tdnn       (128,)           f32
  W_attn       (1536, 128)      f32
  b_attn       (1536,)          f32
and returns the full (32, 3072) f32 output.

Sharding: data-parallel over batch. 8 cores x 4 samples each, weights
replicated.

Key algorithmic move: masked positions contribute EXACTLY zero (the
reference's exp(a - 1e9 - rowmax) underflows to 0.0 in f32), so the host
compacts each sample's time axis to its ~1024 unmasked positions and
zero-pads to a fixed Tp (multiple of 128, 1152 for the seed-0 dataset).
Pad positions have x=0, so e_pad = tanh(b_tdnn) and their logit
a_pad[c] = sum_o W_attn^T[o,c] tanh(b_tdnn)[o] is a per-channel constant;
their S1/S2 contribution is zero and their S0 contribution kb*exp(a_pad)
is precomputed on the host (with the same bf16 weights the device uses)
and subtracted in the tail. This removes the mask matmuls entirely and
scales all per-element engine work by Tp/T ~ 0.56.

Math per sample (all t below over the compacted axis):
  e    = tanh(W_tdnn @ x + b_tdnn)         (BN, Tp)
  a    = W_attn @ e   (b_attn dropped: constant along t, cancels in softmax)
  S0   = sum_t exp(a) - kb*E_pad;  S1 = sum_t exp(a)*x;  S2 = sum_t exp(a)*x^2
  mean = S1/S0;  std = sqrt(clip(S2/S0 - mean^2, 1e-9))

Engine assignment (HW-microbenchmarked per [128,2048] tile: DVE TT mult
bf16 930ns (2x mode), DVE reduce 1365ns, ACT exp+accum 2265ns, ACT
copy+accum 2210ns, Pool TT mult 5393ns):
  ACT:  exp (accum_out gives S0 free) + tanh + A2/12 of the S2 reduces
        (Copy+accum)
  DVE:  p1 = exp*x (+S1 reduce), p2 = p1*x for (12-Q)/12 chunks, the rest
        of the S2 reduces
  Pool: p2 for Q/12 chunks (otherwise idle engine, takes ~1/3 of the
        product work at ~1/3 efficiency)
  PE:   mm1, mm2 (no mask matmuls)
"""

import math

import numpy as np
import ml_dtypes

B, C, T = 32, 1536, 2048
BN = 128
NCORES = 8
SPC = B // NCORES  # samples per core
CK = C // 128      # c chunks of 128 partitions

BF16 = ml_dtypes.bfloat16

# tuning: per sample (12 chunks), Q chunks' p2 on Pool, A2 chunks' S2 on ACT
Q_POOL = 8
A2_ACT = 7

_PROG_CACHE = {}
_LAST_TP = [1152]


def _jblocks(Tp):
    out = []
    off = 0
    while off < Tp:
        w = min(512, Tp - off)
        out.append((off, w))
        off += w
    return out


def _build_program(reps=None, Tp=None, dummy_x=False):
    """Build the per-core program. reps=None: straight-line body.
    reps=K: wrap the whole body in a hardware For_i loop (timing only).
    dummy_x: declare x as Internal DRAM (uninitialized; timing-only builds —
    avoids shipping the large x input over the axon tunnel)."""
    import concourse.bacc as bacc
    import concourse.tile as tile
    import concourse.mybir as mybir
    from contextlib import nullcontext
    from concourse.bass_interp import get_hw_module

    if Tp is None:
        Tp = _LAST_TP[0]

    dt = mybir.dt
    AF = mybir.ActivationFunctionType
    OP = mybir.AluOpType
    JB = _jblocks(Tp)

    nc = bacc.Bacc(
        "TRN2",
        target_bir_lowering=False,
        debug=False,
        num_devices=NCORES,
        num_swdge_queues=4,
    )
    x_d = nc.dram_tensor(
        "x", [SPC, C, Tp], dt.bfloat16,
        kind="Internal" if dummy_x else "ExternalInput",
    )
    wt_d = nc.dram_tensor("wt", [C, BN], dt.bfloat16, kind="ExternalInput")
    wa_d = nc.dram_tensor("wa", [BN, C], dt.bfloat16, kind="ExternalInput")
    bt_d = nc.dram_tensor("bt", [BN, 1], dt.float32, kind="ExternalInput")
    kbe_d = nc.dram_tensor("kbe", [SPC, 128, CK], dt.float32, kind="ExternalInput")
    out_d = nc.dram_tensor("out", [SPC, 2 * C], dt.float32, kind="ExternalOutput")

    with tile.TileContext(nc) as tc:
        with (
            tc.tile_pool(name="const", bufs=1) as constp,
            tc.tile_pool(name="xin", bufs=3 * CK) as xp,
            tc.tile_pool(name="esb", bufs=3) as ep,
            tc.tile_pool(name="expm", bufs=6) as xpm,
            tc.tile_pool(name="prod", bufs=10) as prp,
            tc.tile_pool(name="junk", bufs=3) as jkp,
            tc.tile_pool(name="stats", bufs=1) as statsp,
            tc.tile_pool(name="tail", bufs=2) as tailp,
            tc.tile_pool(name="pse", bufs=2, space="PSUM") as psep,
            tc.tile_pool(name="pa", bufs=2, space="PSUM") as psp,
        ):
            # ---- constants ------------------------------------------------
            wt_sb = constp.tile([128, CK, BN], dt.bfloat16, tag="wt")
            nc.sync.dma_start(
                out=wt_sb, in_=wt_d.ap().rearrange("(k p) o -> p k o", p=128)
            )
            wa_sb = constp.tile([128, C], dt.bfloat16, tag="wa")
            nc.sync.dma_start(out=wa_sb, in_=wa_d.ap())
            bt_sb = constp.tile([128, 1], dt.float32, tag="bt")
            nc.sync.dma_start(out=bt_sb, in_=bt_d.ap())
            kbe_sb = constp.tile([128, SPC, CK], dt.float32, tag="kbe")
            nc.sync.dma_start(
                out=kbe_sb, in_=kbe_d.ap().rearrange("s p k -> p s k")
            )

            loop_cm = tc.For_i(0, reps, 1) if reps is not None else nullcontext()
            with loop_cm:
                stats = []
                for s in range(SPC):
                    S0 = statsp.tile([128, CK], dt.float32, tag=f"S0_{s}")
                    S1 = statsp.tile([128, CK], dt.float32, tag=f"S1_{s}")
                    S2 = statsp.tile([128, CK], dt.float32, tag=f"S2_{s}")
                    stats.append((S0, S1, S2))

                # process samples in pairs; the two chunk streams interleave
                # so the engines always have an independent chunk in flight
                for s0 in range(0, SPC, 2):
                    pair = [s0, s0 + 1]
                    xts = {}
                    esbs = {}
                    for s in pair:
                        for k in range(CK):
                            xt = xp.tile(
                                [128, Tp], dt.bfloat16, tag="x", name=f"x_{s}_{k}"
                            )
                            nc.sync.dma_start(
                                out=xt, in_=x_d.ap()[s, k * 128 : (k + 1) * 128, :]
                            )
                            xts[(s, k)] = xt

                    # mm1 + tanh for both samples of the pair, one 512-block
                    # of PSUM at a time (keeps 3 full-width pa bufs free)
                    for s in pair:
                        e_sb = ep.tile([128, Tp], dt.bfloat16, tag="e", name=f"e_{s}")
                        for bi, (off, w) in enumerate(JB):
                            pse = psep.tile(
                                [128, 512], dt.float32, tag="pse",
                                name=f"pse_{s}_{bi}",
                            )
                            for k in range(CK):
                                nc.tensor.matmul(
                                    pse[:, 0:w],
                                    lhsT=wt_sb[:, k, :],
                                    rhs=xts[(s, k)][:, off : off + w],
                                    start=(k == 0),
                                    stop=(k == CK - 1),
                                )
                            nc.scalar.activation(
                                out=e_sb[:, off : off + w],
                                in_=pse[:, 0:w],
                                func=AF.Tanh,
                                bias=bt_sb,
                                scale=1.0,
                            )
                        esbs[s] = e_sb

                    def s2_stage(s, c, p2):
                        # S2[c] reduce: ACT Copy+accum for A2_ACT of 12
                        # chunks, DVE reduce for the rest
                        if (c % 12) % 2 == 0 or (c % 12) == 1:
                            if A2_ACT >= 7:
                                on_act = True
                            else:
                                on_act = (c % 12) < 2 * A2_ACT
                        else:
                            on_act = False
                        if on_act:
                            junk = jkp.tile(
                                [128, Tp], dt.bfloat16, tag="junk",
                                name=f"junk_{s}_{c}",
                            )
                            nc.scalar.activation(
                                out=junk,
                                in_=p2,
                                func=AF.Copy,
                                accum_out=stats[s][2][:, c : c + 1],
                            )
                        else:
                            nc.vector.tensor_reduce(
                                out=stats[s][2][:, c : c + 1],
                                in_=p2,
                                op=OP.add,
                                axis=mybir.AxisListType.X,
                            )

                    pending = []
                    for c in range(CK):
                        for s in pair:
                            S0, S1, S2 = stats[s]
                            e_sb = esbs[s]
                            expm = xpm.tile(
                                [128, Tp], dt.bfloat16, tag="expm",
                                name=f"expm_{s}_{c}",
                            )
                            pa = psp.tile(
                                [128, Tp], dt.float32, tag="ps", name=f"pa_{s}_{c}"
                            )
                            for off, w in JB:
                                nc.tensor.matmul(
                                    pa[:, off : off + w],
                                    lhsT=wa_sb[:, c * 128 : (c + 1) * 128],
                                    rhs=e_sb[:, off : off + w],
                                    start=True,
                                    stop=True,
                                )
                            nc.scalar.activation(
                                out=expm,
                                in_=pa,
                                func=AF.Exp,
                                accum_out=S0[:, c : c + 1],
                            )
                            p1 = prp.tile(
                                [128, Tp], dt.bfloat16, tag="p1", name=f"p1_{s}_{c}"
                            )
                            nc.vector.tensor_tensor(
                                out=p1, in0=expm, in1=xts[(s, c)], op=OP.mult
                            )
                            nc.vector.tensor_reduce(
                                out=S1[:, c : c + 1],
                                in_=p1,
                                op=OP.add,
                                axis=mybir.AxisListType.X,
                            )
                            p2 = prp.tile(
                                [128, Tp], dt.bfloat16, tag="p2", name=f"p2_{s}_{c}"
                            )
                            # p2 on Pool for Q_POOL of 12 chunks, else DVE
                            if (c % 3 != 2) if Q_POOL == 8 else (c % 12 < Q_POOL):
                                nc.gpsimd.tensor_tensor(
                                    out=p2, in0=p1, in1=xts[(s, c)], op=OP.mult
                                )
                            else:
                                nc.vector.tensor_tensor(
                                    out=p2, in0=p1, in1=xts[(s, c)], op=OP.mult
                                )
                            pending.append((s, c, p2))
                            if len(pending) > 4:
                                s2_stage(*pending.pop(0))
                    for item in pending:
                        s2_stage(*item)

                # ---- tail: mean/std + output DMA --------------------------
                for s in range(SPC):
                    S0, S1, S2 = stats[s]
                    s0c = tailp.tile([128, CK], dt.float32, tag="s0c", name=f"s0c_{s}")
                    nc.vector.tensor_tensor(
                        out=s0c, in0=S0, in1=kbe_sb[:, s, :], op=OP.subtract
                    )
                    r0 = tailp.tile([128, CK], dt.float32, tag="r0", name=f"r0_{s}")
                    nc.vector.reciprocal(out=r0, in_=s0c)
                    mean = tailp.tile(
                        [128, CK], dt.float32, tag="mean", name=f"mean_{s}"
                    )
                    nc.vector.tensor_tensor(out=mean, in0=S1, in1=r0, op=OP.mult)
                    ex2 = tailp.tile([128, CK], dt.float32, tag="ex2", name=f"ex2_{s}")
                    nc.vector.tensor_tensor(out=ex2, in0=S2, in1=r0, op=OP.mult)
                    m2 = tailp.tile([128, CK], dt.float32, tag="m2", name=f"m2_{s}")
                    nc.vector.tensor_tensor(out=m2, in0=mean, in1=mean, op=OP.mult)
                    var = tailp.tile([128, CK], dt.float32, tag="var", name=f"var_{s}")
                    nc.vector.tensor_tensor(out=var, in0=ex2, in1=m2, op=OP.subtract)
                    nc.vector.tensor_scalar(
                        out=var,
                        in0=var,
                        scalar1=1e-9,
                        scalar2=None,
                        op0=OP.max,
                    )
                    std = tailp.tile([128, CK], dt.float32, tag="std", name=f"std_{s}")
                    nc.scalar.activation(out=std, in_=var, func=AF.Sqrt)
                    nc.sync.dma_start(
                        out=out_d.ap()[s, 0:C].rearrange("(ck p) -> p ck", p=128),
                        in_=mean,
                    )
                    nc.sync.dma_start(
                        out=out_d.ap()[s, C : 2 * C].rearrange(
                            "(ck p) -> p ck", p=128
                        ),
                        in_=std,
                    )

    nc.compile()
    nc.m = get_hw_module(nc.m)
    return nc


def _get_program(Tp):
    key = ("nc", Tp)
    if key not in _PROG_CACHE:
        _PROG_CACHE[key] = _build_program(Tp=Tp)
    return _PROG_CACHE[key]


def _prep_inputs(x, padding_mask, W_tdnn, b_tdnn, W_attn, b_attn):
    """Host-side prep: cast/compact/transpose, build per-core input maps."""
    keep = ~np.asarray(padding_mask)
    n_b = keep.sum(axis=1).astype(np.int64)
    Tp = max(512, int(math.ceil(n_b.max() / 128.0) * 128))
    _LAST_TP[0] = Tp

    xb = np.ascontiguousarray(x).astype(BF16)
    xv = xb.view(np.uint16)
    xc = np.zeros((B, C, Tp), dtype=np.uint16)
    for b in range(B):
        idx = np.flatnonzero(keep[b])
        xc[b, :, : idx.size] = xv[b][:, idx]
    xc = xc.view(BF16)

    wt = np.ascontiguousarray(W_tdnn.T).astype(BF16)  # (C, BN)
    wa = np.ascontiguousarray(W_attn.T).astype(BF16)  # (BN, C)
    bt = np.ascontiguousarray(b_tdnn.astype(np.float32).reshape(BN, 1))

    # pad-correction: kb * exp(a_pad) with the same bf16 weights the device
    # uses (pads have x=0 -> e_pad = tanh(b_tdnn))
    e_pad = np.tanh(bt.astype(np.float64))[:, 0].astype(BF16)  # (BN,)
    a_pad = wa.astype(np.float64).T @ e_pad.astype(np.float64)  # (C,)
    E = np.exp(a_pad)  # (C,)
    kb = (Tp - n_b).astype(np.float64)  # (B,)
    # kbe[s, p, k] = kb[s] * E[k*128 + p]
    Em = E.reshape(CK, 128).T  # (128, CK)
    kbe = (kb[:, None, None] * Em[None, :, :]).astype(np.float32)  # (B,128,CK)

    in_maps = []
    for i in range(NCORES):
        sl = slice(i * SPC, (i + 1) * SPC)
        in_maps.append(
            {
                "x": np.ascontiguousarray(xc[sl]),
                "wt": wt,
                "wa": wa,
                "bt": bt,
                "kbe": np.ascontiguousarray(kbe[sl]),
            }
        )
    return in_maps


def kernel(x, padding_mask, W_tdnn, b_tdnn, W_attn, b_attn):
    from concourse.bass_utils import run_bass_kernel_spmd

    in_maps = _prep_inputs(x, padding_mask, W_tdnn, b_tdnn, W_attn, b_attn)
    nc = _get_program(_LAST_TP[0])
    res = run_bass_kernel_spmd(nc, in_maps, core_ids=list(range(NCORES)))
    out = np.concatenate([res.results[i]["out"] for i in range(NCORES)], axis=0)
    return out.astype(np.float32)
